# revision 47
# baseline (speedup 1.0000x reference)
"""FAVOR+ (Performer) attention kernel for Trainium2, 8 NeuronCores.

Math (per batch*head):
    phi_q~[l,m] = exp(arr_q[l,m])                 (g_q, eps, 1/sqrt(m) cancel in num/den ratio)
    phi_k~[l,m] = exp(arr_k[l,m] - g_k[l])
    arr_x = (x / d**0.25) @ proj.T
    g_k[l] = sum_d k[l,d]^2 / (2 sqrt(d))
    ctx[m,e]  = sum_l phi_k~[l,m] v[l,e]
    ksum[m]   = sum_l phi_k~[l,m]
    out[l,e]  = (sum_m phi_q~[l,m] ctx[m,e]) / (sum_m phi_q~[l,m] ksum[m])

Sharding: data-parallel over the 32 (b,h) pairs, 4 per core; projection
matrix replicated. No cross-core communication.

Key structure (all matmul operands in bf16; fp32 PSUM accumulate):
    kT, qT     [d=128, l]    PE transposes (f32), PSUM->SBUF copy rounds to bf16
    arr_k      [l=128, 640]  lhsT=kT chunk bf16; moving projT bf16 in two
                             slices [0:512] and [512:640] (1 cyc/row)
    phi_k      [l=128, 640]  ACT exp, bias=-g_k col, bf16 out
    ctx_aug    [m=128, 129]  x5 chunks, lhsT=phi_k chunk, moving=[1|v] bf16:
                             col 0 accumulates ksum, cols 1:129 ctx. PSUM-
                             accumulated over l. ksum/den need no extra work.
    arr_qT     [m=128, 512]  x5, lhsT=projT chunk, moving=qT (per 512-l group)
    phi_qT     [m, 5, 512]   ACT exp bf16
    nd         [l=128, 129]  lhsT=phi_q chunk [m,l], moving=ctx_aug chunk
                             [m,129] bf16: col 0 = den, cols 1:129 = num
    out        [l, e] = num * recip(den)  via DVE per-partition scalar mult
"""

import sys
import math

if "/opt/trn_rl_repo" not in sys.path:
    sys.path.insert(0, "/opt/trn_rl_repo")

import numpy as np
from contextlib import ExitStack

import concourse.bass as bass
import concourse.bacc as bacc
import concourse.mybir as mybir
import concourse.tile as tile
from concourse.bass_utils import run_bass_kernel_spmd

F32 = mybir.dt.float32
F32R = mybir.dt.float32r
BF16 = mybir.dt.bfloat16
EXP = mybir.ActivationFunctionType.Exp
MULT = mybir.AluOpType.mult
ADD = mybir.AluOpType.add
AXX = mybir.AxisListType.X

B, H, L, D, M = 8, 4, 4096, 128, 640
NCORES = 8
NBH = (B * H) // NCORES  # 4 (b,h) pairs per core
GSCALE = 1.0 / (2.0 * math.sqrt(D))

# ctx_aug chunk offsets in the [128, 1024] PSUM tile (two banks; chunks are
# 129 wide and must not cross the 512-float bank boundary)
CTX_OFF = (0, 129, 258, 512, 641)


def r(ap):
    return ap.bitcast(F32R)


def _emit_ctx(nc, ctx_ps, phik, v_aug, t, gi, ntile):
    """ctx_aug accumulation matmuls for one l-tile.

    start_tensor_calc zeroes a whole 2KB PSUM bank, so only the first matmul
    into each bank (j=0 for bank0, j=3 for bank1) may set start; only the
    last (j=2/j=4) sets stop.
    """
    first = gi == 0
    last = gi == ntile - 1
    for j in range(5):
        nc.tensor.matmul(
            ctx_ps[:, CTX_OFF[j] : CTX_OFF[j] + 129],
            phik[:, 128 * j : 128 * (j + 1)],
            v_aug[:, t, :],
            start=first and j in (0, 3),
            stop=last and j in (2, 4),
        )


def build_bass(n_bh=NBH, seq=L):
    """Builds the per-core Bass program (SPMD: same program on all cores)."""
    nc = bacc.Bacc("TRN2", debug=False)
    q = nc.dram_tensor("q", [n_bh, seq, D], F32, kind="ExternalInput").ap()
    k = nc.dram_tensor("k", [n_bh, seq, D], F32, kind="ExternalInput").ap()
    v = nc.dram_tensor("v", [n_bh, seq, D], F32, kind="ExternalInput").ap()
    projT = nc.dram_tensor("projT", [D, M], BF16, kind="ExternalInput").ap()
    ident = nc.dram_tensor("ident", [128, 128], F32, kind="ExternalInput").ap()
    out = nc.dram_tensor("out", [n_bh, seq, D], F32, kind="ExternalOutput").ap()

    assert seq % 512 == 0
    ngrp = seq // 512  # groups of 4 l-tiles
    ntile = 4 * ngrp

    def ldma(sbuf_tile, dram_ap, g):
        nc.sync.dma_start(
            sbuf_tile[:],
            dram_ap[512 * g : 512 * (g + 1), :].rearrange("(t p) d -> p t d", t=4, p=128),
        )

    with tile.TileContext(nc) as tc, ExitStack() as ctx:
        const = ctx.enter_context(tc.tile_pool(name="const", bufs=1))
        projT_bf = const.tile([D, M], BF16)
        nc.sync.dma_start(projT_bf[:], projT)
        ident_sb = const.tile([128, 128], F32)
        nc.sync.dma_start(ident_sb[:], ident)
        warm = const.tile([128, 1], F32)
        nc.vector.memset(warm[:], 0.0)
        nc.scalar.activation(warm[:], warm[:], EXP, bias=0.0, scale=1.0)

        ld_k = ctx.enter_context(tc.tile_pool(name="ld_k", bufs=3))
        ld_v = ctx.enter_context(tc.tile_pool(name="ld_v", bufs=4))
        ld_q = ctx.enter_context(tc.tile_pool(name="ld_q", bufs=3))
        va_p = ctx.enter_context(tc.tile_pool(name="va", bufs=4))
        kt_p = ctx.enter_context(tc.tile_pool(name="kt_sb", bufs=2))
        qt_p = ctx.enter_context(tc.tile_pool(name="qt_sb", bufs=2))
        phik_p = ctx.enter_context(tc.tile_pool(name="phik", bufs=4))
        phiq_p = ctx.enter_context(tc.tile_pool(name="phiq", bufs=3))
        misc_p = ctx.enter_context(tc.tile_pool(name="misc", bufs=3))
        ctxsb_p = ctx.enter_context(tc.tile_pool(name="ctxsb", bufs=4))
        rc_p = ctx.enter_context(tc.tile_pool(name="recip", bufs=2))
        outsb_p = ctx.enter_context(tc.tile_pool(name="outsb", bufs=2))

        ctx_sbs = []
        for bh in range(n_bh):
            # ---------------- K PASS ----------------
            with tc.tile_pool(name="ps_kt", bufs=2, space="PSUM") as ps_kt, \
                 tc.tile_pool(name="ps_arr", bufs=2, space="PSUM") as ps_arr, \
                 tc.tile_pool(name="ps_ctx", bufs=1, space="PSUM") as ps_ctx:
                prev_k = []
                ctx_ps = ps_ctx.tile([128, 1024], F32, tag="ctxps")
                for g in range(ngrp):
                    k_buf = ld_k.tile([128, 4, D], F32, tag="k")
                    ldma(k_buf, k[bh], g)
                    v_buf = ld_v.tile([128, 4, D], F32, tag="v")
                    ldma(v_buf, v[bh], g)
                    # v_aug = [1 | v] per l-tile, bf16
                    v_aug = va_p.tile([128, 4, 129], BF16, tag="va")
                    nc.gpsimd.memset(v_aug[:, :, 0:1], 1.0)
                    nc.gpsimd.tensor_copy(v_aug[:, :, 1:129], v_buf[:])
                    # negb[:, t] = -sum_d k^2 / (2 sqrt d)
                    sq = misc_p.tile([128, 4, D], F32, tag="sq")
                    nc.gpsimd.tensor_mul(sq[:], k_buf[:], k_buf[:])
                    negb = misc_p.tile([128, 4], F32, tag="negb")
                    nc.vector.tensor_reduce(negb[:], sq[:], axis=AXX, op=ADD, negate=True)
                    nc.vector.tensor_scalar_mul(negb[:], negb[:], GSCALE)
                    kt_ps = ps_kt.tile([128, 512], F32, tag="kt")
                    for t in range(4):
                        nc.tensor.transpose(
                            kt_ps[:, 128 * t : 128 * (t + 1)],
                            k_buf[:, t, :],
                            ident_sb[:],
                        )
                    kt_sb = kt_p.tile([128, 512], BF16, tag="kt")
                    nc.vector.tensor_copy(kt_sb[:], kt_ps[:])
                    for t in range(4):
                        gi = 4 * g + t
                        arr = ps_arr.tile([128, 1024], F32, tag="arr")
                        lhsT = kt_sb[:, 128 * t : 128 * (t + 1)]
                        nc.tensor.matmul(arr[:, 0:512], lhsT, projT_bf[:, 0:512])
                        nc.tensor.matmul(arr[:, 512:640], lhsT, projT_bf[:, 512:640])
                        phik = phik_p.tile([128, M], BF16, tag="phik")
                        nc.scalar.activation(
                            phik[:], arr[:, 0:M], EXP, bias=negb[:, t : t + 1], scale=1.0
                        )
                        # software pipeline: arr/exp run two tiles ahead of
                        # the ctx matmuls so the exp input is always ready the
                        # moment ACT frees up (hides the arr->ACT sem hop).
                        prev_k.append((ctx_ps, phik, v_aug, t, gi))
                        if len(prev_k) > 2:
                            _emit_ctx(nc, *prev_k.pop(0), ntile)
                # flush this bh's pending ctx matmuls, then snapshot ctx_aug
                # [m, 5, 129] bf16 (col 0 of each chunk = ksum) to SBUF
                for args in prev_k:
                    _emit_ctx(nc, *args, ntile)
                prev_k = []
                ctx_sb = ctxsb_p.tile([128, 5, 129], BF16, tag="ctx")
                nc.vector.tensor_copy(
                    ctx_sb[:, 0:3, :],
                    ctx_ps[:, 0:387].rearrange("p (c x) -> p c x", c=3, x=129),
                )
                nc.vector.tensor_copy(
                    ctx_sb[:, 3:5, :],
                    ctx_ps[:, 512:770].rearrange("p (c x) -> p c x", c=2, x=129),
                )
                ctx_sbs.append(ctx_sb)

            # ---------------- Q PASS ----------------
            # Software-pipelined: group g's qt/arr_q/exp are issued first,
            # then the nd/divide/store stage for group g-1, so the PE work
            # feeding the next exp is never queued behind nd matmuls that
            # wait on the current exp. arrq lives in two single-buffered
            # pools (chunks 0-1 / 2-4) so WAR on the tile rotation only
            # orders against the matching exp call.
            ctx_sb = ctx_sbs[bh]
            with tc.tile_pool(name="ps_qt", bufs=1, space="PSUM") as ps_qt, \
                 tc.tile_pool(name="ps_arrqA", bufs=1, space="PSUM") as ps_arrqA, \
                 tc.tile_pool(name="ps_arrqB", bufs=1, space="PSUM") as ps_arrqB, \
                 tc.tile_pool(name="ps_nd", bufs=2, space="PSUM") as ps_nd:
                prev_q = None
                for g in range(ngrp + 1):
                    cur = None
                    if g < ngrp:
                        q_buf = ld_q.tile([128, 4, D], F32, tag="q")
                        ldma(q_buf, q[bh], g)
                        qt_ps = ps_qt.tile([128, 512], F32, tag="qt")
                        for t in range(4):
                            nc.tensor.transpose(
                                qt_ps[:, 128 * t : 128 * (t + 1)],
                                q_buf[:, t, :],
                                ident_sb[:],
                            )
                        qt_sb = qt_p.tile([128, 512], BF16, tag="qt")
                        nc.vector.tensor_copy(qt_sb[:], qt_ps[:])
                        arrqA = ps_arrqA.tile([128, 2, 512], F32, tag="arrqA")
                        arrqB = ps_arrqB.tile([128, 3, 512], F32, tag="arrqB")
                        phiq = phiq_p.tile([128, 5, 512], BF16, tag="phiq")
                        for j in range(2):
                            nc.tensor.matmul(
                                arrqA[:, j, :],
                                projT_bf[:, 128 * j : 128 * (j + 1)],
                                qt_sb[:],
                            )
                        nc.scalar.activation(
                            phiq[:, 0:2, :], arrqA[:], EXP, bias=0.0, scale=1.0
                        )
                        for j in range(2, 5):
                            nc.tensor.matmul(
                                arrqB[:, j - 2, :],
                                projT_bf[:, 128 * j : 128 * (j + 1)],
                                qt_sb[:],
                            )
                        nc.scalar.activation(
                            phiq[:, 2:5, :], arrqB[:], EXP, bias=0.0, scale=1.0
                        )
                        cur = (phiq, g)
                    if prev_q is not None:
                        phiq_pv, gp = prev_q
                        out_sb = outsb_p.tile([128, 4, D], F32, tag="out")
                        recip = rc_p.tile([128, 4], F32, tag="recip")
                        for half in range(2):
                            nd = ps_nd.tile([128, 512], F32, tag="nd")
                            # both 129-wide chunks share one PSUM bank: single
                            # start (first matmul) / stop (last) per bank
                            for ci in range(2):
                                c = 2 * half + ci
                                off = 256 * ci
                                for j in range(5):
                                    nc.tensor.matmul(
                                        nd[:, off : off + 129],
                                        phiq_pv[:, j, 128 * c : 128 * (c + 1)],
                                        ctx_sb[:, j, :],
                                        start=(ci == 0 and j == 0),
                                        stop=(ci == 1 and j == 4),
                                    )
                            # den cols {0, 256} -> recip[:, 2*half : 2*half+2]
                            nc.vector.reciprocal(
                                recip[:, 2 * half : 2 * half + 2].rearrange(
                                    "p (a b) -> p a b", a=2, b=1
                                ),
                                nd[:].rearrange("p (a b) -> p a b", a=2, b=256)[
                                    :, :, 0:1
                                ],
                            )
                            for ci in range(2):
                                c = 2 * half + ci
                                off = 256 * ci
                                nc.vector.tensor_scalar(
                                    out_sb[:, c, :],
                                    nd[:, off + 1 : off + 129],
                                    recip[:, c : c + 1],
                                    None,
                                    MULT,
                                )
                        nc.sync.dma_start(
                            out[bh, 512 * gp : 512 * (gp + 1), :].rearrange(
                                "(t p) d -> p t d", t=4, p=128
                            ),
                            out_sb[:],
                        )
                    prev_q = cur
    nc.compile()
    return nc


_NC_CACHE = {}


def _get_nc(n_bh=NBH, seq=L):
    key = (n_bh, seq)
    if key not in _NC_CACHE:
        _NC_CACHE[key] = build_bass(n_bh, seq)
    return _NC_CACHE[key]


def host_inputs(projection_matrix):
    import ml_dtypes

    projT_f = np.ascontiguousarray(
        (np.asarray(projection_matrix, dtype=np.float32) / (D**0.25)).T
    ).astype(ml_dtypes.bfloat16)
    ident = np.eye(128, dtype=np.float32)
    return projT_f, ident


def kernel(q, k, v, projection_matrix, _trace=False, _trace_kwargs=None):
    q = np.ascontiguousarray(np.asarray(q, dtype=np.float32)).reshape(B * H, L, D)
    k = np.ascontiguousarray(np.asarray(k, dtype=np.float32)).reshape(B * H, L, D)
    v = np.ascontiguousarray(np.asarray(v, dtype=np.float32)).reshape(B * H, L, D)
    projT_f, ident = host_inputs(projection_matrix)

    in_maps = []
    for c in range(NCORES):
        sl = slice(NBH * c, NBH * (c + 1))
        in_maps.append(
            {
                "q": np.ascontiguousarray(q[sl]),
                "k": np.ascontiguousarray(k[sl]),
                "v": np.ascontiguousarray(v[sl]),
                "projT": projT_f,
                "ident": ident,
            }
        )

    nc = _get_nc()
    kwargs = {}
    if _trace:
        kwargs["trace"] = True
        kwargs.update(_trace_kwargs or {})
    res = run_bass_kernel_spmd(nc, in_maps, core_ids=list(range(NCORES)), **kwargs)
    outs = np.concatenate([res.results[c]["out"] for c in range(NCORES)], axis=0)
    result = outs.reshape(B, H, L, D).astype(np.float32)
    if _trace:
        return result, res
    return result


def timed_run(q, k, v, projection_matrix, iters=5):
    """Steady-state wall timing of the NEFF execution via PJRT with
    device-resident inputs (upper bound on HW exec: includes dispatch)."""
    import time
    import jax
    from jax.sharding import Mesh, PartitionSpec
    from jax.experimental.shard_map import shard_map
    from concourse import bass2jax

    q = np.ascontiguousarray(np.asarray(q, dtype=np.float32)).reshape(B * H, L, D)
    k = np.ascontiguousarray(np.asarray(k, dtype=np.float32)).reshape(B * H, L, D)
    v = np.ascontiguousarray(np.asarray(v, dtype=np.float32)).reshape(B * H, L, D)
    projT_f, ident = host_inputs(projection_matrix)
    nc = _get_nc()
    bass2jax.install_neuronx_cc_hook()

    in_names = []
    out_names = []
    out_avals = []
    zero_outs = []
    import concourse.mybir as mybir_

    partition_name = nc.partition_id_tensor.name if nc.partition_id_tensor else None
    for alloc in nc.m.functions[0].allocations:
        if not isinstance(alloc, mybir_.MemoryLocationSet):
            continue
        name = alloc.memorylocations[0].name
        if alloc.kind == "ExternalInput":
            if name != partition_name:
                in_names.append(name)
        elif alloc.kind == "ExternalOutput":
            out_names.append(name)
            shape = list(alloc.tensor_shape)
            out_avals.append(jax.core.ShapedArray(shape, np.float32))
            zero_outs.append(np.zeros(shape, np.float32))
    n_params = len(in_names)
    n_outs = len(out_names)
    all_names = in_names + out_names
    if partition_name is not None:
        all_names = all_names + [partition_name]

    def _body(*args):
        operands = list(args)
        if partition_name is not None:
            operands.append(bass2jax.partition_id_tensor())
        outs = bass2jax._bass_exec_p.bind(
            *operands,
            out_avals=tuple(out_avals),
            in_names=tuple(all_names),
            out_names=tuple(out_names),
            lowering_input_output_aliases=(),
            sim_require_finite=True,
            sim_require_nnan=True,
            nc=nc,
        )
        return tuple(outs)

    devices = jax.devices()[:NCORES]
    mesh = Mesh(np.asarray(devices), ("core",))
    in_specs = (PartitionSpec("core"),) * (n_params + n_outs)
    out_specs = (PartitionSpec("core"),) * n_outs
    sharded = jax.jit(
        shard_map(_body, mesh=mesh, in_specs=in_specs, out_specs=out_specs, check_rep=False),
        keep_unused=True,
    )

    per_core_vals = {
        "q": [q[NBH * c : NBH * (c + 1)] for c in range(NCORES)],
        "k": [k[NBH * c : NBH * (c + 1)] for c in range(NCORES)],
        "v": [v[NBH * c : NBH * (c + 1)] for c in range(NCORES)],
        "projT": [projT_f] * NCORES,
        "ident": [ident] * NCORES,
    }
    concat_in = [
        np.concatenate(per_core_vals[nm], axis=0) for nm in in_names
    ]
    concat_zeros = [
        np.zeros((NCORES * z.shape[0], *z.shape[1:]), z.dtype) for z in zero_outs
    ]
    sharding = jax.sharding.NamedSharding(mesh, PartitionSpec("core"))
    dev_in = [jax.device_put(a, sharding) for a in concat_in]
    dev_zero = [jax.device_put(a, sharding) for a in concat_zeros]
    # warm-up (compile + first exec)
    r0 = sharded(*dev_in, *dev_zero)
    jax.block_until_ready(r0)
    times = []
    for _ in range(iters):
        t0 = time.perf_counter()
        rr = sharded(*dev_in, *dev_zero)
        jax.block_until_ready(rr)
        times.append(time.perf_counter() - t0)
    out = np.asarray(rr[out_names.index("out")]).reshape(NCORES, NBH, L, D)
    result = out.reshape(B, H, L, D)
    return result, times


# revision 53
# speedup vs baseline: 1.0046x; 1.0046x over previous
"""FAVOR+ (Performer) attention kernel for Trainium2, 8 NeuronCores.

Math (per batch*head):
    phi_q~[l,m] = exp(arr_q[l,m])                 (g_q, eps, 1/sqrt(m) cancel in num/den ratio)
    phi_k~[l,m] = exp(arr_k[l,m] - g_k[l])
    arr_x = (x / d**0.25) @ proj.T
    g_k[l] = sum_d k[l,d]^2 / (2 sqrt(d))
    ctx[m,e]  = sum_l phi_k~[l,m] v[l,e]
    ksum[m]   = sum_l phi_k~[l,m]
    out[l,e]  = (sum_m phi_q~[l,m] ctx[m,e]) / (sum_m phi_q~[l,m] ksum[m])

Sharding: data-parallel over the 32 (b,h) pairs, 4 per core; projection
matrix replicated. No cross-core communication.

Key structure (all matmul operands in bf16; fp32 PSUM accumulate):
    kT, qT     [d=128, l]    PE transposes (f32), PSUM->SBUF copy rounds to bf16
    arr_k      [l=128, 640]  lhsT=kT chunk bf16; moving projT bf16 in two
                             slices [0:512] and [512:640] (1 cyc/row)
    phi_k      [l=128, 640]  ACT exp, bias=-g_k col, bf16 out
    ctx_aug    [m=128, 129]  x5 chunks, lhsT=phi_k chunk, moving=[1|v] bf16:
                             col 0 accumulates ksum, cols 1:129 ctx. PSUM-
                             accumulated over l. ksum/den need no extra work.
    arr_qT     [m=128, 512]  x5, lhsT=projT chunk, moving=qT (per 512-l group)
    phi_qT     [m, 5, 512]   ACT exp bf16
    nd         [l=128, 129]  lhsT=phi_q chunk [m,l], moving=ctx_aug chunk
                             [m,129] bf16: col 0 = den, cols 1:129 = num
    out        [l, e] = num * recip(den)  via DVE per-partition scalar mult
"""

import sys
import math

if "/opt/trn_rl_repo" not in sys.path:
    sys.path.insert(0, "/opt/trn_rl_repo")

import numpy as np
from contextlib import ExitStack

import concourse.bass as bass
import concourse.bacc as bacc
import concourse.mybir as mybir
import concourse.tile as tile
from concourse.bass_utils import run_bass_kernel_spmd

F32 = mybir.dt.float32
F32R = mybir.dt.float32r
BF16 = mybir.dt.bfloat16
EXP = mybir.ActivationFunctionType.Exp
MULT = mybir.AluOpType.mult
ADD = mybir.AluOpType.add
AXX = mybir.AxisListType.X

B, H, L, D, M = 8, 4, 4096, 128, 640
NCORES = 8
NBH = (B * H) // NCORES  # 4 (b,h) pairs per core
GSCALE = 1.0 / (2.0 * math.sqrt(D))

# ctx_aug chunk offsets in the [128, 1024] PSUM tile (two banks; chunks are
# 129 wide and must not cross the 512-float bank boundary)
CTX_OFF = (0, 129, 258, 512, 641)


def r(ap):
    return ap.bitcast(F32R)


def _emit_ctx(nc, ctx_ps, phik, v_aug, t, gi, ntile):
    """ctx_aug accumulation matmuls for one l-tile.

    start_tensor_calc zeroes a whole 2KB PSUM bank, so only the first matmul
    into each bank (j=0 for bank0, j=3 for bank1) may set start; only the
    last (j=2/j=4) sets stop.
    """
    first = gi == 0
    last = gi == ntile - 1
    for j in range(5):
        nc.tensor.matmul(
            ctx_ps[:, CTX_OFF[j] : CTX_OFF[j] + 129],
            phik[:, 128 * j : 128 * (j + 1)],
            v_aug[:, t, :],
            start=first and j in (0, 3),
            stop=last and j in (2, 4),
        )


def build_bass(n_bh=NBH, seq=L):
    """Builds the per-core Bass program (SPMD: same program on all cores)."""
    nc = bacc.Bacc("TRN2", debug=False)
    q = nc.dram_tensor("q", [n_bh, seq, D], F32, kind="ExternalInput").ap()
    k = nc.dram_tensor("k", [n_bh, seq, D], F32, kind="ExternalInput").ap()
    v = nc.dram_tensor("v", [n_bh, seq, D], F32, kind="ExternalInput").ap()
    projT = nc.dram_tensor("projT", [D, M], BF16, kind="ExternalInput").ap()
    ident = nc.dram_tensor("ident", [128, 128], F32, kind="ExternalInput").ap()
    out = nc.dram_tensor("out", [n_bh, seq, D], F32, kind="ExternalOutput").ap()

    assert seq % 512 == 0
    ngrp = seq // 512  # groups of 4 l-tiles
    ntile = 4 * ngrp

    def ldma(sbuf_tile, dram_ap, g):
        nc.sync.dma_start(
            sbuf_tile[:],
            dram_ap[512 * g : 512 * (g + 1), :].rearrange("(t p) d -> p t d", t=4, p=128),
        )

    with tile.TileContext(nc) as tc, ExitStack() as ctx:
        const = ctx.enter_context(tc.tile_pool(name="const", bufs=1))
        ld_k = ctx.enter_context(tc.tile_pool(name="ld_k", bufs=3))
        ld_v = ctx.enter_context(tc.tile_pool(name="ld_v", bufs=4))
        ld_q = ctx.enter_context(tc.tile_pool(name="ld_q", bufs=3))

        # DMA order tuned for the startup critical chain: k first (feeds
        # squares + transposes), then ident (transposes), projT (first arr),
        # and v last (only needed by the lagged ctx stage)
        k_buf0 = ld_k.tile([128, 4, D], F32, tag="k")
        ldma(k_buf0, k[0], 0)
        ident_sb = const.tile([128, 128], F32)
        nc.sync.dma_start(ident_sb[:], ident)
        projT_bf = const.tile([D, M], BF16)
        nc.sync.dma_start(projT_bf[:], projT)
        v_buf0 = ld_v.tile([128, 4, D], F32, tag="v")
        ldma(v_buf0, v[0], 0)
        kv0 = (k_buf0, v_buf0)
        warm = const.tile([128, 1], F32)
        nc.vector.memset(warm[:], 0.0)
        nc.scalar.activation(warm[:], warm[:], EXP, bias=0.0, scale=1.0)
        va_p = ctx.enter_context(tc.tile_pool(name="va", bufs=4))
        kt_p = ctx.enter_context(tc.tile_pool(name="kt_sb", bufs=2))
        qt_p = ctx.enter_context(tc.tile_pool(name="qt_sb", bufs=2))
        phik_p = ctx.enter_context(tc.tile_pool(name="phik", bufs=4))
        phiq_p = ctx.enter_context(tc.tile_pool(name="phiq", bufs=3))
        misc_p = ctx.enter_context(tc.tile_pool(name="misc", bufs=3))
        ctxsb_p = ctx.enter_context(tc.tile_pool(name="ctxsb", bufs=4))
        rc_p = ctx.enter_context(tc.tile_pool(name="recip", bufs=2))
        outsb_p = ctx.enter_context(tc.tile_pool(name="outsb", bufs=2))

        ctx_sbs = []
        for bh in range(n_bh):
            # ---------------- K PASS ----------------
            with tc.tile_pool(name="ps_kt", bufs=2, space="PSUM") as ps_kt, \
                 tc.tile_pool(name="ps_arr", bufs=2, space="PSUM") as ps_arr, \
                 tc.tile_pool(name="ps_ctx", bufs=1, space="PSUM") as ps_ctx:
                prev_k = []
                ctx_ps = ps_ctx.tile([128, 1024], F32, tag="ctxps")
                for g in range(ngrp):
                    if bh == 0 and g == 0:
                        k_buf, v_buf = kv0
                    else:
                        k_buf = ld_k.tile([128, 4, D], F32, tag="k")
                        ldma(k_buf, k[bh], g)
                        v_buf = ld_v.tile([128, 4, D], F32, tag="v")
                        ldma(v_buf, v[bh], g)
                    # v_aug = [1 | v] per l-tile, bf16
                    v_aug = va_p.tile([128, 4, 129], BF16, tag="va")
                    nc.gpsimd.memset(v_aug[:, :, 0:1], 1.0)
                    nc.gpsimd.tensor_copy(v_aug[:, :, 1:129], v_buf[:])
                    # negb[:, t] = -sum_d k^2 / (2 sqrt d)
                    sq = misc_p.tile([128, 4, D], F32, tag="sq")
                    nc.gpsimd.tensor_mul(sq[:], k_buf[:], k_buf[:])
                    negb = misc_p.tile([128, 4], F32, tag="negb")
                    nc.vector.tensor_reduce(negb[:], sq[:], axis=AXX, op=ADD, negate=True)
                    nc.vector.tensor_scalar_mul(negb[:], negb[:], GSCALE)
                    kt_ps = ps_kt.tile([128, 512], F32, tag="kt")
                    for t in range(4):
                        nc.tensor.transpose(
                            kt_ps[:, 128 * t : 128 * (t + 1)],
                            k_buf[:, t, :],
                            ident_sb[:],
                        )
                    kt_sb = kt_p.tile([128, 512], BF16, tag="kt")
                    nc.vector.tensor_copy(kt_sb[:], kt_ps[:])
                    for t in range(4):
                        gi = 4 * g + t
                        arr = ps_arr.tile([128, 1024], F32, tag="arr")
                        lhsT = kt_sb[:, 128 * t : 128 * (t + 1)]
                        nc.tensor.matmul(arr[:, 0:512], lhsT, projT_bf[:, 0:512])
                        nc.tensor.matmul(arr[:, 512:640], lhsT, projT_bf[:, 512:640])
                        phik = phik_p.tile([128, M], BF16, tag="phik")
                        nc.scalar.activation(
                            phik[:], arr[:, 0:M], EXP, bias=negb[:, t : t + 1], scale=1.0
                        )
                        # software pipeline: arr/exp run two tiles ahead of
                        # the ctx matmuls so the exp input is always ready the
                        # moment ACT frees up (hides the arr->ACT sem hop).
                        prev_k.append((ctx_ps, phik, v_aug, t, gi))
                        if len(prev_k) > 2:
                            _emit_ctx(nc, *prev_k.pop(0), ntile)
                # flush this bh's pending ctx matmuls, then snapshot ctx_aug
                # [m, 5, 129] bf16 (col 0 of each chunk = ksum) to SBUF
                for args in prev_k:
                    _emit_ctx(nc, *args, ntile)
                prev_k = []
                ctx_sb = ctxsb_p.tile([128, 5, 129], BF16, tag="ctx")
                nc.vector.tensor_copy(
                    ctx_sb[:, 0:3, :],
                    ctx_ps[:, 0:387].rearrange("p (c x) -> p c x", c=3, x=129),
                )
                nc.vector.tensor_copy(
                    ctx_sb[:, 3:5, :],
                    ctx_ps[:, 512:770].rearrange("p (c x) -> p c x", c=2, x=129),
                )
                ctx_sbs.append(ctx_sb)

            # ---------------- Q PASS ----------------
            # Software-pipelined: group g's qt/arr_q/exp are issued first,
            # then the nd/divide/store stage for group g-1, so the PE work
            # feeding the next exp is never queued behind nd matmuls that
            # wait on the current exp. arrq lives in two single-buffered
            # pools (chunks 0-1 / 2-4) so WAR on the tile rotation only
            # orders against the matching exp call.
            ctx_sb = ctx_sbs[bh]
            with tc.tile_pool(name="ps_qt", bufs=1, space="PSUM") as ps_qt, \
                 tc.tile_pool(name="ps_arrqA", bufs=1, space="PSUM") as ps_arrqA, \
                 tc.tile_pool(name="ps_arrqB", bufs=1, space="PSUM") as ps_arrqB, \
                 tc.tile_pool(name="ps_nd", bufs=2, space="PSUM") as ps_nd:
                prev_q = None
                for g in range(ngrp + 1):
                    cur = None
                    if g < ngrp:
                        q_buf = ld_q.tile([128, 4, D], F32, tag="q")
                        ldma(q_buf, q[bh], g)
                        qt_ps = ps_qt.tile([128, 512], F32, tag="qt")
                        for t in range(4):
                            nc.tensor.transpose(
                                qt_ps[:, 128 * t : 128 * (t + 1)],
                                q_buf[:, t, :],
                                ident_sb[:],
                            )
                        qt_sb = qt_p.tile([128, 512], BF16, tag="qt")
                        nc.vector.tensor_copy(qt_sb[:], qt_ps[:])
                        arrqA = ps_arrqA.tile([128, 2, 512], F32, tag="arrqA")
                        arrqB = ps_arrqB.tile([128, 3, 512], F32, tag="arrqB")
                        phiq = phiq_p.tile([128, 5, 512], BF16, tag="phiq")
                        for j in range(2):
                            nc.tensor.matmul(
                                arrqA[:, j, :],
                                projT_bf[:, 128 * j : 128 * (j + 1)],
                                qt_sb[:],
                            )
                        nc.scalar.activation(
                            phiq[:, 0:2, :], arrqA[:], EXP, bias=0.0, scale=1.0
                        )
                        for j in range(2, 5):
                            nc.tensor.matmul(
                                arrqB[:, j - 2, :],
                                projT_bf[:, 128 * j : 128 * (j + 1)],
                                qt_sb[:],
                            )
                        nc.scalar.activation(
                            phiq[:, 2:5, :], arrqB[:], EXP, bias=0.0, scale=1.0
                        )
                        cur = (phiq, g)
                    if prev_q is not None:
                        phiq_pv, gp = prev_q
                        out_sb = outsb_p.tile([128, 4, D], F32, tag="out")
                        recip = rc_p.tile([128, 4], F32, tag="recip")
                        for half in range(2):
                            nd = ps_nd.tile([128, 512], F32, tag="nd")
                            # both 129-wide chunks share one PSUM bank: single
                            # start (first matmul) / stop (last) per bank
                            for ci in range(2):
                                c = 2 * half + ci
                                off = 256 * ci
                                for j in range(5):
                                    nc.tensor.matmul(
                                        nd[:, off : off + 129],
                                        phiq_pv[:, j, 128 * c : 128 * (c + 1)],
                                        ctx_sb[:, j, :],
                                        start=(ci == 0 and j == 0),
                                        stop=(ci == 1 and j == 4),
                                    )
                            # den cols {0, 256} -> recip[:, 2*half : 2*half+2]
                            nc.vector.reciprocal(
                                recip[:, 2 * half : 2 * half + 2].rearrange(
                                    "p (a b) -> p a b", a=2, b=1
                                ),
                                nd[:].rearrange("p (a b) -> p a b", a=2, b=256)[
                                    :, :, 0:1
                                ],
                            )
                            for ci in range(2):
                                c = 2 * half + ci
                                off = 256 * ci
                                nc.vector.tensor_scalar(
                                    out_sb[:, c, :],
                                    nd[:, off + 1 : off + 129],
                                    recip[:, c : c + 1],
                                    None,
                                    MULT,
                                )
                        nc.sync.dma_start(
                            out[bh, 512 * gp : 512 * (gp + 1), :].rearrange(
                                "(t p) d -> p t d", t=4, p=128
                            ),
                            out_sb[:],
                        )
                    prev_q = cur
    nc.compile()
    return nc


_NC_CACHE = {}


def _get_nc(n_bh=NBH, seq=L):
    key = (n_bh, seq)
    if key not in _NC_CACHE:
        _NC_CACHE[key] = build_bass(n_bh, seq)
    return _NC_CACHE[key]


def host_inputs(projection_matrix):
    import ml_dtypes

    projT_f = np.ascontiguousarray(
        (np.asarray(projection_matrix, dtype=np.float32) / (D**0.25)).T
    ).astype(ml_dtypes.bfloat16)
    ident = np.eye(128, dtype=np.float32)
    return projT_f, ident


def kernel(q, k, v, projection_matrix, _trace=False, _trace_kwargs=None):
    q = np.ascontiguousarray(np.asarray(q, dtype=np.float32)).reshape(B * H, L, D)
    k = np.ascontiguousarray(np.asarray(k, dtype=np.float32)).reshape(B * H, L, D)
    v = np.ascontiguousarray(np.asarray(v, dtype=np.float32)).reshape(B * H, L, D)
    projT_f, ident = host_inputs(projection_matrix)

    in_maps = []
    for c in range(NCORES):
        sl = slice(NBH * c, NBH * (c + 1))
        in_maps.append(
            {
                "q": np.ascontiguousarray(q[sl]),
                "k": np.ascontiguousarray(k[sl]),
                "v": np.ascontiguousarray(v[sl]),
                "projT": projT_f,
                "ident": ident,
            }
        )

    nc = _get_nc()
    kwargs = {}
    if _trace:
        kwargs["trace"] = True
        kwargs.update(_trace_kwargs or {})
    res = run_bass_kernel_spmd(nc, in_maps, core_ids=list(range(NCORES)), **kwargs)
    outs = np.concatenate([res.results[c]["out"] for c in range(NCORES)], axis=0)
    result = outs.reshape(B, H, L, D).astype(np.float32)
    if _trace:
        return result, res
    return result


def timed_run(q, k, v, projection_matrix, iters=5):
    """Steady-state wall timing of the NEFF execution via PJRT with
    device-resident inputs (upper bound on HW exec: includes dispatch)."""
    import time
    import jax
    from jax.sharding import Mesh, PartitionSpec
    from jax.experimental.shard_map import shard_map
    from concourse import bass2jax

    q = np.ascontiguousarray(np.asarray(q, dtype=np.float32)).reshape(B * H, L, D)
    k = np.ascontiguousarray(np.asarray(k, dtype=np.float32)).reshape(B * H, L, D)
    v = np.ascontiguousarray(np.asarray(v, dtype=np.float32)).reshape(B * H, L, D)
    projT_f, ident = host_inputs(projection_matrix)
    nc = _get_nc()
    bass2jax.install_neuronx_cc_hook()

    in_names = []
    out_names = []
    out_avals = []
    zero_outs = []
    import concourse.mybir as mybir_

    partition_name = nc.partition_id_tensor.name if nc.partition_id_tensor else None
    for alloc in nc.m.functions[0].allocations:
        if not isinstance(alloc, mybir_.MemoryLocationSet):
            continue
        name = alloc.memorylocations[0].name
        if alloc.kind == "ExternalInput":
            if name != partition_name:
                in_names.append(name)
        elif alloc.kind == "ExternalOutput":
            out_names.append(name)
            shape = list(alloc.tensor_shape)
            out_avals.append(jax.core.ShapedArray(shape, np.float32))
            zero_outs.append(np.zeros(shape, np.float32))
    n_params = len(in_names)
    n_outs = len(out_names)
    all_names = in_names + out_names
    if partition_name is not None:
        all_names = all_names + [partition_name]

    def _body(*args):
        operands = list(args)
        if partition_name is not None:
            operands.append(bass2jax.partition_id_tensor())
        outs = bass2jax._bass_exec_p.bind(
            *operands,
            out_avals=tuple(out_avals),
            in_names=tuple(all_names),
            out_names=tuple(out_names),
            lowering_input_output_aliases=(),
            sim_require_finite=True,
            sim_require_nnan=True,
            nc=nc,
        )
        return tuple(outs)

    devices = jax.devices()[:NCORES]
    mesh = Mesh(np.asarray(devices), ("core",))
    in_specs = (PartitionSpec("core"),) * (n_params + n_outs)
    out_specs = (PartitionSpec("core"),) * n_outs
    sharded = jax.jit(
        shard_map(_body, mesh=mesh, in_specs=in_specs, out_specs=out_specs, check_rep=False),
        keep_unused=True,
    )

    per_core_vals = {
        "q": [q[NBH * c : NBH * (c + 1)] for c in range(NCORES)],
        "k": [k[NBH * c : NBH * (c + 1)] for c in range(NCORES)],
        "v": [v[NBH * c : NBH * (c + 1)] for c in range(NCORES)],
        "projT": [projT_f] * NCORES,
        "ident": [ident] * NCORES,
    }
    concat_in = [
        np.concatenate(per_core_vals[nm], axis=0) for nm in in_names
    ]
    concat_zeros = [
        np.zeros((NCORES * z.shape[0], *z.shape[1:]), z.dtype) for z in zero_outs
    ]
    sharding = jax.sharding.NamedSharding(mesh, PartitionSpec("core"))
    dev_in = [jax.device_put(a, sharding) for a in concat_in]
    dev_zero = [jax.device_put(a, sharding) for a in concat_zeros]
    # warm-up (compile + first exec)
    r0 = sharded(*dev_in, *dev_zero)
    jax.block_until_ready(r0)
    times = []
    for _ in range(iters):
        t0 = time.perf_counter()
        rr = sharded(*dev_in, *dev_zero)
        jax.block_until_ready(rr)
        times.append(time.perf_counter() - t0)
    out = np.asarray(rr[out_names.index("out")]).reshape(NCORES, NBH, L, D)
    result = out.reshape(B, H, L, D)
    return result, times


# revision 67
# speedup vs baseline: 1.0192x; 1.0145x over previous
"""FAVOR+ (Performer) attention kernel for Trainium2, 8 NeuronCores.

Math (per batch*head):
    phi_q~[l,m] = exp(arr_q[l,m])                 (g_q, eps, 1/sqrt(m) cancel in num/den ratio)
    phi_k~[l,m] = exp(arr_k[l,m] - g_k[l])
    arr_x = (x / d**0.25) @ proj.T
    g_k[l] = sum_d k[l,d]^2 / (2 sqrt(d))
    ctx[m,e]  = sum_l phi_k~[l,m] v[l,e]
    ksum[m]   = sum_l phi_k~[l,m]
    out[l,e]  = (sum_m phi_q~[l,m] ctx[m,e]) / (sum_m phi_q~[l,m] ksum[m])

Sharding: data-parallel over the 32 (b,h) pairs, 4 per core; projection
matrix replicated. No cross-core communication.

Key structure (all matmul operands in bf16; fp32 PSUM accumulate):
    kT, qT     [d=128, l]    PE transposes (f32), PSUM->SBUF copy rounds to bf16
    arr_k      [l=128, 640]  lhsT=kT chunk bf16; moving projT bf16 in two
                             slices [0:512] and [512:640] (1 cyc/row)
    phi_k      [l=128, 640]  ACT exp, bias=-g_k col, bf16 out
    ctx_aug    [m=128, 129]  x5 chunks, lhsT=phi_k chunk, moving=[1|v] bf16:
                             col 0 accumulates ksum, cols 1:129 ctx. PSUM-
                             accumulated over l. ksum/den need no extra work.
    arr_qT     [m=128, 512]  x5, lhsT=projT chunk, moving=qT (per 512-l group)
    phi_qT     [m, 5, 512]   ACT exp bf16
    nd         [l=128, 129]  lhsT=phi_q chunk [m,l], moving=ctx_aug chunk
                             [m,129] bf16: col 0 = den, cols 1:129 = num
    out        [l, e] = num * recip(den)  via DVE per-partition scalar mult
"""

import sys
import math

if "/opt/trn_rl_repo" not in sys.path:
    sys.path.insert(0, "/opt/trn_rl_repo")

import numpy as np
from contextlib import ExitStack

import concourse.bass as bass
import concourse.bacc as bacc
import concourse.mybir as mybir
import concourse.tile as tile
from concourse.bass_utils import run_bass_kernel_spmd

F32 = mybir.dt.float32
F32R = mybir.dt.float32r
BF16 = mybir.dt.bfloat16
EXP = mybir.ActivationFunctionType.Exp
MULT = mybir.AluOpType.mult
ADD = mybir.AluOpType.add
AXX = mybir.AxisListType.X

B, H, L, D, M = 8, 4, 4096, 128, 640
NCORES = 8
NBH = (B * H) // NCORES  # 4 (b,h) pairs per core
GSCALE = 1.0 / (2.0 * math.sqrt(D))

# ctx_aug chunk offsets in the [128, 1024] PSUM tile (two banks; chunks are
# 129 wide and must not cross the 512-float bank boundary)
CTX_OFF = (0, 129, 258, 512, 641)


def r(ap):
    return ap.bitcast(F32R)


def _emit_ctx(nc, ctx_ps, phik, v_aug, t, gi, ntile):
    """ctx_aug accumulation matmuls for one l-tile.

    start_tensor_calc zeroes a whole 2KB PSUM bank, so only the first matmul
    into each bank (j=0 for bank0, j=3 for bank1) may set start; only the
    last (j=2/j=4) sets stop.
    """
    first = gi == 0
    last = gi == ntile - 1
    for j in range(5):
        nc.tensor.matmul(
            ctx_ps[:, CTX_OFF[j] : CTX_OFF[j] + 129],
            phik[:, 128 * j : 128 * (j + 1)],
            v_aug[:, t, :],
            start=first and j in (0, 3),
            stop=last and j in (2, 4),
        )


def build_bass(n_bh=NBH, seq=L):
    """Builds the per-core Bass program (SPMD: same program on all cores)."""
    nc = bacc.Bacc("TRN2", debug=False)
    q = nc.dram_tensor("q", [n_bh, seq, D], F32, kind="ExternalInput").ap()
    k = nc.dram_tensor("k", [n_bh, seq, D], F32, kind="ExternalInput").ap()
    v = nc.dram_tensor("v", [n_bh, seq, D], F32, kind="ExternalInput").ap()
    projT = nc.dram_tensor("projT", [D, M], BF16, kind="ExternalInput").ap()
    ident = nc.dram_tensor("ident", [128, 128], F32, kind="ExternalInput").ap()
    out = nc.dram_tensor("out", [n_bh, seq, D], F32, kind="ExternalOutput").ap()

    assert seq % 512 == 0
    ngrp = seq // 512  # groups of 4 l-tiles
    ntile = 4 * ngrp

    def ldma(sbuf_tile, dram_ap, g):
        nc.sync.dma_start(
            sbuf_tile[:],
            dram_ap[512 * g : 512 * (g + 1), :].rearrange("(t p) d -> p t d", t=4, p=128),
        )

    with tile.TileContext(nc) as tc, ExitStack() as ctx:
        const = ctx.enter_context(tc.tile_pool(name="const", bufs=1))
        ld_k = ctx.enter_context(tc.tile_pool(name="ld_k", bufs=3))
        ld_v = ctx.enter_context(tc.tile_pool(name="ld_v", bufs=4))
        ld_q = ctx.enter_context(tc.tile_pool(name="ld_q", bufs=3))

        # DMA order tuned for the startup critical chain: k first (feeds
        # squares + transposes), then ident (transposes), projT (first arr),
        # and v last (only needed by the lagged ctx stage)
        k_buf0 = ld_k.tile([128, 4, D], F32, tag="k")
        ldma(k_buf0, k[0], 0)
        ident_sb = const.tile([128, 128], F32)
        nc.sync.dma_start(ident_sb[:], ident)
        projT_bf = const.tile([D, M], BF16)
        nc.sync.dma_start(projT_bf[:], projT)
        v_buf0 = ld_v.tile([128, 4, D], F32, tag="v")
        ldma(v_buf0, v[0], 0)
        kv0 = (k_buf0, v_buf0)
        warm = const.tile([128, 1], F32)
        nc.vector.memset(warm[:], 0.0)
        nc.scalar.activation(warm[:], warm[:], EXP, bias=0.0, scale=1.0)
        va_p = ctx.enter_context(tc.tile_pool(name="va", bufs=4))
        kt_p = ctx.enter_context(tc.tile_pool(name="kt_sb", bufs=2))
        qt_p = ctx.enter_context(tc.tile_pool(name="qt_sb", bufs=2))
        phik_p = ctx.enter_context(tc.tile_pool(name="phik", bufs=5))
        phiq_p = ctx.enter_context(tc.tile_pool(name="phiq", bufs=3))
        misc_p = ctx.enter_context(tc.tile_pool(name="misc", bufs=3))
        ctxsb_p = ctx.enter_context(tc.tile_pool(name="ctxsb", bufs=4))
        rc_p = ctx.enter_context(tc.tile_pool(name="recip", bufs=2))
        outsb_p = ctx.enter_context(tc.tile_pool(name="outsb", bufs=2))

        ctx_sbs = []
        for bh in range(n_bh):
            # ---------------- K PASS ----------------
            with tc.tile_pool(name="ps_kt", bufs=2, space="PSUM") as ps_kt, \
                 tc.tile_pool(name="ps_arr", bufs=2, space="PSUM") as ps_arr, \
                 tc.tile_pool(name="ps_ctx", bufs=1, space="PSUM") as ps_ctx:
                prev_k = []
                ctx_ps = ps_ctx.tile([128, 1024], F32, tag="ctxps")
                for g in range(ngrp):
                    if bh == 0 and g == 0:
                        k_buf, v_buf = kv0
                    else:
                        k_buf = ld_k.tile([128, 4, D], F32, tag="k")
                        ldma(k_buf, k[bh], g)
                        v_buf = ld_v.tile([128, 4, D], F32, tag="v")
                        ldma(v_buf, v[bh], g)
                    # v_aug = [1 | v] per l-tile, bf16
                    v_aug = va_p.tile([128, 4, 129], BF16, tag="va")
                    nc.gpsimd.memset(v_aug[:, :, 0:1], 1.0)
                    nc.gpsimd.tensor_copy(v_aug[:, :, 1:129], v_buf[:])
                    # negb[:, t] = -sum_d k^2 / (2 sqrt d)
                    sq = misc_p.tile([128, 4, D], F32, tag="sq")
                    if bh == 0 and g == 0:
                        # DVE is idle at program start and ~2x faster per elem
                        nc.vector.tensor_mul(sq[:], k_buf[:], k_buf[:])
                    else:
                        nc.gpsimd.tensor_mul(sq[:], k_buf[:], k_buf[:])
                    negb = misc_p.tile([128, 4], F32, tag="negb")
                    nc.vector.tensor_reduce(negb[:], sq[:], axis=AXX, op=ADD, negate=True)
                    nc.vector.tensor_scalar_mul(negb[:], negb[:], GSCALE)
                    kt_ps = ps_kt.tile([128, 512], F32, tag="kt")
                    for t in range(4):
                        nc.tensor.transpose(
                            kt_ps[:, 128 * t : 128 * (t + 1)],
                            k_buf[:, t, :],
                            ident_sb[:],
                        )
                    kt_sb = kt_p.tile([128, 512], BF16, tag="kt")
                    nc.vector.tensor_copy(kt_sb[:], kt_ps[:])
                    for t in range(4):
                        gi = 4 * g + t
                        arr = ps_arr.tile([128, 1024], F32, tag="arr")
                        lhsT = kt_sb[:, 128 * t : 128 * (t + 1)]
                        nc.tensor.matmul(arr[:, 0:512], lhsT, projT_bf[:, 0:512])
                        nc.tensor.matmul(arr[:, 512:640], lhsT, projT_bf[:, 512:640])
                        phik = phik_p.tile([128, M], BF16, tag="phik")
                        nc.scalar.activation(
                            phik[:], arr[:, 0:M], EXP, bias=negb[:, t : t + 1], scale=1.0
                        )
                        # software pipeline: arr/exp run two tiles ahead of
                        # the ctx matmuls so the exp input is always ready the
                        # moment ACT frees up (hides the arr->ACT sem hop).
                        prev_k.append((ctx_ps, phik, v_aug, t, gi))
                        if len(prev_k) > 2:
                            _emit_ctx(nc, *prev_k.pop(0), ntile)
                # flush this bh's pending ctx matmuls, then snapshot ctx_aug
                # [m, 5, 129] bf16 (col 0 of each chunk = ksum) to SBUF
                for args in prev_k:
                    _emit_ctx(nc, *args, ntile)
                prev_k = []
                ctx_sb = ctxsb_p.tile([128, 5, 129], BF16, tag="ctx")
                nc.vector.tensor_copy(
                    ctx_sb[:, 0:3, :],
                    ctx_ps[:, 0:387].rearrange("p (c x) -> p c x", c=3, x=129),
                )
                nc.vector.tensor_copy(
                    ctx_sb[:, 3:5, :],
                    ctx_ps[:, 512:770].rearrange("p (c x) -> p c x", c=2, x=129),
                )
                ctx_sbs.append(ctx_sb)

            # ---------------- Q PASS ----------------
            # Software-pipelined: group g's qt/arr_q/exp are issued first,
            # then the nd/divide/store stage for group g-1, so the PE work
            # feeding the next exp is never queued behind nd matmuls that
            # wait on the current exp. arrq lives in two single-buffered
            # pools (chunks 0-1 / 2-4) so WAR on the tile rotation only
            # orders against the matching exp call.
            ctx_sb = ctx_sbs[bh]
            with tc.tile_pool(name="ps_qt", bufs=1, space="PSUM") as ps_qt, \
                 tc.tile_pool(name="ps_arrqA", bufs=1, space="PSUM") as ps_arrqA, \
                 tc.tile_pool(name="ps_arrqB", bufs=1, space="PSUM") as ps_arrqB, \
                 tc.tile_pool(name="ps_nd", bufs=2, space="PSUM") as ps_nd:
                prev_q = None
                for g in range(ngrp + 1):
                    cur = None
                    if g < ngrp:
                        q_buf = ld_q.tile([128, 4, D], F32, tag="q")
                        ldma(q_buf, q[bh], g)
                        qt_ps = ps_qt.tile([128, 512], F32, tag="qt")
                        for t in range(4):
                            nc.tensor.transpose(
                                qt_ps[:, 128 * t : 128 * (t + 1)],
                                q_buf[:, t, :],
                                ident_sb[:],
                            )
                        qt_sb = qt_p.tile([128, 512], BF16, tag="qt")
                        nc.vector.tensor_copy(qt_sb[:], qt_ps[:])
                        arrqA = ps_arrqA.tile([128, 3, 512], F32, tag="arrqA")
                        arrqB = ps_arrqB.tile([128, 2, 512], F32, tag="arrqB")
                        phiq = phiq_p.tile([128, 5, 512], BF16, tag="phiq")
                        for j in range(3):
                            nc.tensor.matmul(
                                arrqA[:, j, :],
                                projT_bf[:, 128 * j : 128 * (j + 1)],
                                qt_sb[:],
                            )
                        nc.scalar.activation(
                            phiq[:, 0:3, :], arrqA[:], EXP, bias=0.0, scale=1.0
                        )
                        for j in range(3, 5):
                            nc.tensor.matmul(
                                arrqB[:, j - 3, :],
                                projT_bf[:, 128 * j : 128 * (j + 1)],
                                qt_sb[:],
                            )
                        nc.scalar.activation(
                            phiq[:, 3:5, :], arrqB[:], EXP, bias=0.0, scale=1.0
                        )
                        cur = (phiq, g)
                    if prev_q is not None:
                        phiq_pv, gp = prev_q
                        out_sb = outsb_p.tile([128, 4, D], F32, tag="out")
                        recip = rc_p.tile([128, 4], F32, tag="recip")
                        for half in range(2):
                            nd = ps_nd.tile([128, 512], F32, tag="nd")
                            # both 129-wide chunks share one PSUM bank: single
                            # start (first matmul) / stop (last) per bank
                            for ci in range(2):
                                c = 2 * half + ci
                                off = 256 * ci
                                for j in range(5):
                                    nc.tensor.matmul(
                                        nd[:, off : off + 129],
                                        phiq_pv[:, j, 128 * c : 128 * (c + 1)],
                                        ctx_sb[:, j, :],
                                        start=(ci == 0 and j == 0),
                                        stop=(ci == 1 and j == 4),
                                    )
                            # den cols {0, 256} -> recip[:, 2*half : 2*half+2]
                            nc.vector.reciprocal(
                                recip[:, 2 * half : 2 * half + 2].rearrange(
                                    "p (a b) -> p a b", a=2, b=1
                                ),
                                nd[:].rearrange("p (a b) -> p a b", a=2, b=256)[
                                    :, :, 0:1
                                ],
                            )
                            for ci in range(2):
                                c = 2 * half + ci
                                off = 256 * ci
                                nc.vector.tensor_scalar(
                                    out_sb[:, c, :],
                                    nd[:, off + 1 : off + 129],
                                    recip[:, c : c + 1],
                                    None,
                                    MULT,
                                )
                        last = bh == n_bh - 1 and gp == ngrp - 1
                        if last:
                            # drain shortcut: ship each half as soon as it is
                            # divided instead of one DMA at the very end
                            for hh in range(2):
                                nc.sync.dma_start(
                                    out[
                                        bh,
                                        512 * gp + 256 * hh : 512 * gp + 256 * (hh + 1),
                                        :,
                                    ].rearrange("(t p) d -> p t d", t=2, p=128),
                                    out_sb[:, 2 * hh : 2 * (hh + 1), :],
                                )
                        else:
                            nc.sync.dma_start(
                                out[bh, 512 * gp : 512 * (gp + 1), :].rearrange(
                                    "(t p) d -> p t d", t=4, p=128
                                ),
                                out_sb[:],
                            )
                    prev_q = cur
    nc.compile()
    return nc


_NC_CACHE = {}


def _get_nc(n_bh=NBH, seq=L):
    key = (n_bh, seq)
    if key not in _NC_CACHE:
        _NC_CACHE[key] = build_bass(n_bh, seq)
    return _NC_CACHE[key]


def host_inputs(projection_matrix):
    import ml_dtypes

    projT_f = np.ascontiguousarray(
        (np.asarray(projection_matrix, dtype=np.float32) / (D**0.25)).T
    ).astype(ml_dtypes.bfloat16)
    ident = np.eye(128, dtype=np.float32)
    return projT_f, ident


def kernel(q, k, v, projection_matrix, _trace=False, _trace_kwargs=None):
    q = np.ascontiguousarray(np.asarray(q, dtype=np.float32)).reshape(B * H, L, D)
    k = np.ascontiguousarray(np.asarray(k, dtype=np.float32)).reshape(B * H, L, D)
    v = np.ascontiguousarray(np.asarray(v, dtype=np.float32)).reshape(B * H, L, D)
    projT_f, ident = host_inputs(projection_matrix)

    in_maps = []
    for c in range(NCORES):
        sl = slice(NBH * c, NBH * (c + 1))
        in_maps.append(
            {
                "q": np.ascontiguousarray(q[sl]),
                "k": np.ascontiguousarray(k[sl]),
                "v": np.ascontiguousarray(v[sl]),
                "projT": projT_f,
                "ident": ident,
            }
        )

    nc = _get_nc()
    kwargs = {}
    if _trace:
        kwargs["trace"] = True
        kwargs.update(_trace_kwargs or {})
    res = run_bass_kernel_spmd(nc, in_maps, core_ids=list(range(NCORES)), **kwargs)
    outs = np.concatenate([res.results[c]["out"] for c in range(NCORES)], axis=0)
    result = outs.reshape(B, H, L, D).astype(np.float32)
    if _trace:
        return result, res
    return result


def timed_run(q, k, v, projection_matrix, iters=5):
    """Steady-state wall timing of the NEFF execution via PJRT with
    device-resident inputs (upper bound on HW exec: includes dispatch)."""
    import time
    import jax
    from jax.sharding import Mesh, PartitionSpec
    from jax.experimental.shard_map import shard_map
    from concourse import bass2jax

    q = np.ascontiguousarray(np.asarray(q, dtype=np.float32)).reshape(B * H, L, D)
    k = np.ascontiguousarray(np.asarray(k, dtype=np.float32)).reshape(B * H, L, D)
    v = np.ascontiguousarray(np.asarray(v, dtype=np.float32)).reshape(B * H, L, D)
    projT_f, ident = host_inputs(projection_matrix)
    nc = _get_nc()
    bass2jax.install_neuronx_cc_hook()

    in_names = []
    out_names = []
    out_avals = []
    zero_outs = []
    import concourse.mybir as mybir_

    partition_name = nc.partition_id_tensor.name if nc.partition_id_tensor else None
    for alloc in nc.m.functions[0].allocations:
        if not isinstance(alloc, mybir_.MemoryLocationSet):
            continue
        name = alloc.memorylocations[0].name
        if alloc.kind == "ExternalInput":
            if name != partition_name:
                in_names.append(name)
        elif alloc.kind == "ExternalOutput":
            out_names.append(name)
            shape = list(alloc.tensor_shape)
            out_avals.append(jax.core.ShapedArray(shape, np.float32))
            zero_outs.append(np.zeros(shape, np.float32))
    n_params = len(in_names)
    n_outs = len(out_names)
    all_names = in_names + out_names
    if partition_name is not None:
        all_names = all_names + [partition_name]

    def _body(*args):
        operands = list(args)
        if partition_name is not None:
            operands.append(bass2jax.partition_id_tensor())
        outs = bass2jax._bass_exec_p.bind(
            *operands,
            out_avals=tuple(out_avals),
            in_names=tuple(all_names),
            out_names=tuple(out_names),
            lowering_input_output_aliases=(),
            sim_require_finite=True,
            sim_require_nnan=True,
            nc=nc,
        )
        return tuple(outs)

    devices = jax.devices()[:NCORES]
    mesh = Mesh(np.asarray(devices), ("core",))
    in_specs = (PartitionSpec("core"),) * (n_params + n_outs)
    out_specs = (PartitionSpec("core"),) * n_outs
    sharded = jax.jit(
        shard_map(_body, mesh=mesh, in_specs=in_specs, out_specs=out_specs, check_rep=False),
        keep_unused=True,
    )

    per_core_vals = {
        "q": [q[NBH * c : NBH * (c + 1)] for c in range(NCORES)],
        "k": [k[NBH * c : NBH * (c + 1)] for c in range(NCORES)],
        "v": [v[NBH * c : NBH * (c + 1)] for c in range(NCORES)],
        "projT": [projT_f] * NCORES,
        "ident": [ident] * NCORES,
    }
    concat_in = [
        np.concatenate(per_core_vals[nm], axis=0) for nm in in_names
    ]
    concat_zeros = [
        np.zeros((NCORES * z.shape[0], *z.shape[1:]), z.dtype) for z in zero_outs
    ]
    sharding = jax.sharding.NamedSharding(mesh, PartitionSpec("core"))
    dev_in = [jax.device_put(a, sharding) for a in concat_in]
    dev_zero = [jax.device_put(a, sharding) for a in concat_zeros]
    # warm-up (compile + first exec)
    r0 = sharded(*dev_in, *dev_zero)
    jax.block_until_ready(r0)
    times = []
    for _ in range(iters):
        t0 = time.perf_counter()
        rr = sharded(*dev_in, *dev_zero)
        jax.block_until_ready(rr)
        times.append(time.perf_counter() - t0)
    out = np.asarray(rr[out_names.index("out")]).reshape(NCORES, NBH, L, D)
    result = out.reshape(B, H, L, D)
    return result, times
